# revision 8
# baseline (speedup 1.0000x reference)
"""Trainium2 Bass kernel for nn_DiffeqSolver (RK4 ODE solve, 2-layer tanh MLP drift).

Strategy (data-parallel across 8 NeuronCores):
  - Shard the 32768 latent rows (NTRAJ*B*N) across 8 cores -> 4096 rows/core.
  - Everything on-chip is feature-major ("transposed"): y^T [64, rows], so both
    matmuls of the MLP have their contraction dim on SBUF partitions.
  - Per RK4 stage i: z = W1^T y_i^T (2 matmuls, M-blocks of 128, into one merged
    PSUM tile [128, 2, 512]), a = tanh(z) (single wide ACT op), then
    P_i = (s_i W2)^T a (2 accumulating matmuls, K-blocks of 128) where the RK4
    step-size factors s_i in (h/2, h/2, h, h/6) are folded into host-prescaled
    copies of W2.  Stage states: y_{i+1} = y + P_i (one fused DVE op).
    Final combine: y_next = (y2 + 2*y3 + y4 - y)/3 + P4 (GpSimd+DVE fused ops).
  - Output is written transposed ([steps, 64, rows] per core); the host
    re-transposes while gathering.
"""

import sys

if "/opt/trn_rl_repo" not in sys.path:
    sys.path.insert(0, "/opt/trn_rl_repo")

import numpy as np

_NCORES = 8
_T = 32
_NTRAJ, _B, _N, _L = 1, 32, 1024, 64
_H = 256
_ROWS = _NTRAJ * _B * _N          # 32768 total latent rows
_R = _ROWS // _NCORES             # 4096 rows per core
_WT = 512                         # column-tile width (matmul moving-dim)
_NT = _R // _WT                   # 8 column tiles per core

_BUILD_CACHE = {}


def _build(nsteps: int, n_hslots: int, b1_nonzero: bool, b2_nonzero: bool,
           repeat: int = 1):
    import concourse.mybir as mybir
    import concourse.tile as tile
    from concourse import bacc

    f32 = mybir.dt.float32
    f32r = mybir.dt.float32r
    Alu = mybir.AluOpType
    Act = mybir.ActivationFunctionType

    nc = bacc.Bacc("TRN2", target_bir_lowering=False, debug=False,
                   num_devices=_NCORES)

    y0t = nc.dram_tensor("y0t", [_L, _R], f32r, kind="ExternalInput")
    w1d = nc.dram_tensor("w1d", [_L, _H], f32r, kind="ExternalInput")
    # Host-prescaled W2 variants: [128, slot, variant(h/2, h, h/6), kblock, 64]
    w2d = nc.dram_tensor("w2d", [128, n_hslots, 3, 2, _L], f32r,
                         kind="ExternalInput")
    b1d = (nc.dram_tensor("b1d", [128, 2], f32, kind="ExternalInput")
           if b1_nonzero else None)
    # b2 scaled by s_i per variant, plus a 4th column holding 3*(h/6)*b2
    b2d = (nc.dram_tensor("b2d", [_L, n_hslots, 4], f32, kind="ExternalInput")
           if b2_nonzero else None)
    outt = nc.dram_tensor("outt", [nsteps, _L, _R], f32r, kind="ExternalOutput")

    with tile.TileContext(nc) as tc:
        with (
            tc.tile_pool(name="singles", bufs=1) as singles,
            tc.tile_pool(name="zpool", bufs=3, space="PSUM") as zpool,
            tc.tile_pool(name="ppool", bufs=2, space="PSUM") as ppool,
            tc.tile_pool(name="apool", bufs=3) as apool,
            tc.tile_pool(name="ypool", bufs=12) as ypool,
            tc.tile_pool(name="gpool", bufs=8) as gpool,
        ):
            ybuf = [singles.tile([_L, _R], f32r, tag="ybuf0", name="ybuf0"),
                    singles.tile([_L, _R], f32r, tag="ybuf1", name="ybuf1")]
            w1sb = singles.tile([_L, _H], f32r, tag="w1sb")
            w2sb = singles.tile([128, n_hslots, 3, 2, _L], f32r, tag="w2sb")
            nc.sync.dma_start(out=ybuf[0][:, :], in_=y0t.ap())
            nc.sync.dma_start(out=w1sb[:, :], in_=w1d.ap())
            nc.sync.dma_start(out=w2sb[:, :, :, :, :], in_=w2d.ap())
            if b1_nonzero:
                b1sb = singles.tile([128, 2], f32, tag="b1sb")
                nc.sync.dma_start(out=b1sb[:, :], in_=b1d.ap())
            if b2_nonzero:
                b2sb = singles.tile([_L, n_hslots, 4], f32, tag="b2sb")
                nc.sync.dma_start(out=b2sb[:, :, :], in_=b2d.ap())

            for s in range(nsteps * repeat):
                s = s % nsteps
                slot = 0 if n_hslots == 1 else s
                ycur = ybuf[s % 2]
                ynxt = ybuf[(s + 1) % 2]
                # Wavefront emission: stage-outer, tiles-inner, so each
                # engine's (in-order) stream holds 8 independent tiles per
                # stage and pipelines fill.
                ysls = [ycur[:, t * _WT:(t + 1) * _WT] for t in range(_NT)]
                prev = list(ysls)
                ystage = [[] for _ in range(_NT)]
                for e in range(4):
                    v = 0 if e < 2 else (1 if e == 2 else 2)
                    for t in range(_NT):
                        ysl = ysls[t]
                        z = zpool.tile([128, 2, _WT], f32, tag="z")
                        nc.tensor.matmul(z[:, 0], w1sb[:, 0:128], prev[t],
                                         start=True, stop=True)
                        nc.tensor.matmul(z[:, 1], w1sb[:, 128:256], prev[t],
                                         start=True, stop=True)
                        a = apool.tile([128, 2, _WT], f32r, tag="a")
                        if b1_nonzero:
                            nc.scalar.activation(a[:, 0], z[:, 0], Act.Tanh,
                                                 bias=b1sb[:, 0])
                            nc.scalar.activation(a[:, 1], z[:, 1], Act.Tanh,
                                                 bias=b1sb[:, 1])
                        else:
                            nc.scalar.activation(a[:, :, :], z[:, :, :],
                                                 Act.Tanh)
                        p = ppool.tile([_L, _WT], f32, tag="p")
                        nc.tensor.matmul(p[:, :], w2sb[:, slot, v, 0], a[:, 0],
                                         start=True, stop=False)
                        nc.tensor.matmul(p[:, :], w2sb[:, slot, v, 1], a[:, 1],
                                         start=False, stop=True)
                        if e < 3:
                            yn = ypool.tile([_L, _WT], f32r, tag=f"y{e}")
                            if b2_nonzero:
                                nc.vector.scalar_tensor_tensor(
                                    yn[:, :], p[:, :], b2sb[:, slot, v],
                                    ysl, Alu.add, Alu.add)
                            else:
                                nc.vector.tensor_add(yn[:, :], p[:, :], ysl)
                            ystage[t].append(yn)
                            prev[t] = yn[:, :]
                        else:
                            y2v, y3v, y4v = ystage[t]
                            g0 = gpool.tile([_L, _WT], f32, tag="g0")
                            nc.gpsimd.tensor_scalar_mul(g0[:, :], y3v[:, :],
                                                        2.0)
                            g1 = gpool.tile([_L, _WT], f32, tag="g1")
                            nc.gpsimd.tensor_add(g1[:, :], g0[:, :],
                                                 y2v[:, :])
                            g2 = gpool.tile([_L, _WT], f32, tag="g2")
                            if b2_nonzero:
                                nc.vector.scalar_tensor_tensor(
                                    g2[:, :], y4v[:, :], b2sb[:, slot, 3],
                                    g1[:, :], Alu.add, Alu.add)
                            else:
                                nc.vector.tensor_add(g2[:, :], g1[:, :],
                                                     y4v[:, :])
                            d1 = gpool.tile([_L, _WT], f32, tag="d1")
                            nc.vector.tensor_sub(d1[:, :], g2[:, :], ysl)
                            nc.vector.scalar_tensor_tensor(
                                ynxt[:, t * _WT:(t + 1) * _WT], d1[:, :],
                                1.0 / 3.0, p[:, :], Alu.mult, Alu.add)
                nc.sync.dma_start(out=outt.ap()[s], in_=ynxt[:, :])

    nc.compile()
    return nc


def _prep_inputs(first_point, time_steps_to_predict, W1, b1, W2, b2):
    """Host-side shard + transpose + weight prescale. Returns (key, in_maps, nsteps)."""
    fp = np.ascontiguousarray(np.asarray(first_point, dtype=np.float32))
    ts = np.asarray(time_steps_to_predict, dtype=np.float32)
    W1 = np.ascontiguousarray(np.asarray(W1, dtype=np.float32))
    W2 = np.ascontiguousarray(np.asarray(W2, dtype=np.float32))
    b1 = np.asarray(b1, dtype=np.float32)
    b2 = np.asarray(b2, dtype=np.float32)

    nsteps = int(ts.shape[0]) - 1
    hs = np.diff(ts.astype(np.float64)).astype(np.float32)      # [nsteps]
    uniform = bool(np.all(hs == hs[0]))
    n_hslots = 1 if uniform else nsteps
    hs_used = hs[:1] if uniform else hs

    b1_nonzero = bool(np.any(b1))
    b2_nonzero = bool(np.any(b2))

    flat = fp.reshape(_ROWS, _L)

    # W2 as [128 partitions, kblock, 64], scaled per (slot, variant)
    w2kb = W2.reshape(2, 128, _L).transpose(1, 0, 2)            # [128, 2, 64]
    scales = np.stack([hs_used / 2.0, hs_used, hs_used / 6.0], axis=1)  # [S,3]
    w2s = (scales[None, :, :, None, None] *
           w2kb[:, None, None, :, :]).astype(np.float32)        # [128,S,3,2,64]
    w2s = np.ascontiguousarray(w2s)

    in_maps = []
    for c in range(_NCORES):
        shard = flat[c * _R:(c + 1) * _R]                       # [R, 64]
        y0t = np.ascontiguousarray(shard.T)                     # [64, R]
        m = {"y0t": y0t, "w1d": W1, "w2d": w2s}
        if b1_nonzero:
            m["b1d"] = np.ascontiguousarray(b1.reshape(2, 128).T)
        if b2_nonzero:
            b2s = np.empty((_L, n_hslots, 4), np.float32)
            b2s[:, :, 0] = b2[:, None] * (hs_used / 2.0)[None, :]
            b2s[:, :, 1] = b2[:, None] * hs_used[None, :]
            b2s[:, :, 2] = b2[:, None] * (hs_used / 6.0)[None, :]
            b2s[:, :, 3] = b2[:, None] * (hs_used / 2.0)[None, :]  # 3*(h/6)
            m["b2d"] = b2s
        in_maps.append(m)

    key = (nsteps, n_hslots, b1_nonzero, b2_nonzero)
    return key, in_maps, nsteps


def get_nc(first_point, time_steps_to_predict, W1, b1, W2, b2):
    """Build (or fetch cached) the compiled Bass program for these inputs."""
    key, in_maps, nsteps = _prep_inputs(
        first_point, time_steps_to_predict, W1, b1, W2, b2)
    if key not in _BUILD_CACHE:
        _BUILD_CACHE[key] = _build(*key)
    return _BUILD_CACHE[key], in_maps, nsteps


def _assemble(first_point, core_outs, nsteps):
    """core_outs: list of [nsteps, 64, R] per core -> full [1, T, B, N, L]."""
    fp = np.asarray(first_point, dtype=np.float32)
    out = np.empty((_NTRAJ, nsteps + 1, _B, _N, _L), np.float32)
    out[:, 0] = fp
    bs = _B // _NCORES                                          # batches/core
    for c in range(_NCORES):
        dev = core_outs[c]                                      # [S, 64, R]
        # -> [S, R, 64] -> [S, bs, N, L]
        out[0, 1:, c * bs:(c + 1) * bs] = dev.transpose(0, 2, 1).reshape(
            nsteps, bs, _N, _L)
    return out


def kernel(first_point, time_steps_to_predict, W1, b1, W2, b2):
    from concourse.bass_utils import run_bass_kernel_spmd

    nc, in_maps, nsteps = get_nc(
        first_point, time_steps_to_predict, W1, b1, W2, b2)
    res = run_bass_kernel_spmd(nc, in_maps, core_ids=list(range(_NCORES)))
    core_outs = [res.results[c]["outt"] for c in range(_NCORES)]
    return _assemble(first_point, core_outs, nsteps)


# revision 10
# speedup vs baseline: 1.2581x; 1.2581x over previous
"""Trainium2 Bass kernel for nn_DiffeqSolver (RK4 ODE solve, 2-layer tanh MLP drift).

Strategy (data-parallel across 8 NeuronCores):
  - Shard the 32768 latent rows (NTRAJ*B*N) across 8 cores -> 4096 rows/core.
  - On-chip everything is feature-major: y^T [64, rows].  Rows are split into
    two halves packed on SBUF partitions 0-63 (rows 0..2047) and 64-127
    (rows 2048..4095), so elementwise ops run on all 128 lanes and the two
    halves' matmuls run concurrently in separate PE row/column groups.
  - Matmuls are bf16 (weights + stage inputs); PSUM accumulation and the
    persistent state stay fp32.  bf16 rounding only enters through the
    h-scaled drift k_i, so state error stays ~1e-4/step.
  - Per RK4 stage i: z = W1^T y_i^T (row-tiled pairs, PSUM [128,2,512] per
    half), a = tanh(z) (one wide ACT op per half), P_i = (s_i W2)^T a
    (col-tiled pairs accumulating into one PSUM [128,512] tile) with RK4
    factors s_i = (h/2, h/2, h, h/6) folded into host-prescaled W2 copies.
  - y_{i+1} = y + P_i (one DVE op, bf16 out, feeds next stage's matmul).
    Final combine in full fp32 from the PSUMs:
      y_next = y + (P1 + 2 P2 + P3)/3 + P4.
  - Output is written transposed ([steps, 128, 2048] per core); the host
    unpacks halves and re-transposes while gathering.
"""

import sys

if "/opt/trn_rl_repo" not in sys.path:
    sys.path.insert(0, "/opt/trn_rl_repo")

import numpy as np
import ml_dtypes

_NCORES = 8
_T = 32
_NTRAJ, _B, _N, _L = 1, 32, 1024, 64
_H = 256
_ROWS = _NTRAJ * _B * _N          # 32768 total latent rows
_R = _ROWS // _NCORES             # 4096 rows per core
_RH = _R // 2                     # 2048 rows per partition-half
_WT = 512                         # column-tile width (matmul moving-dim)
_NT = _RH // _WT                  # 4 column tiles per step

_BUILD_CACHE = {}


def _build(nsteps: int, n_hslots: int, b1_nonzero: bool, b2_nonzero: bool,
           repeat: int = 1, slim: bool = False):
    import concourse.mybir as mybir
    import concourse.tile as tile
    from concourse import bacc

    f32 = mybir.dt.float32
    bf16 = mybir.dt.bfloat16
    Alu = mybir.AluOpType
    Act = mybir.ActivationFunctionType

    nc = bacc.Bacc("TRN2", target_bir_lowering=False, debug=False,
                   num_devices=_NCORES)

    y0f = nc.dram_tensor("y0f", [128, _RH], f32, kind="ExternalInput")
    y0b = nc.dram_tensor("y0b", [128, _RH], bf16, kind="ExternalInput")
    w1d = nc.dram_tensor("w1d", [128, _H], bf16, kind="ExternalInput")
    # Host-prescaled W2 variants: [128, slot, variant(h/2, h, h/6), kblock, 64]
    w2d = nc.dram_tensor("w2d", [128, n_hslots, 3, 2, _L], bf16,
                         kind="ExternalInput")
    b1d = (nc.dram_tensor("b1d", [128, 2], f32, kind="ExternalInput")
           if b1_nonzero else None)
    # b2 scaled by (h/2, h, h/6) per variant, partition-halves duplicated
    b2d = (nc.dram_tensor("b2d", [128, n_hslots, 3], f32, kind="ExternalInput")
           if b2_nonzero else None)
    if slim:
        outt = nc.dram_tensor("outt", [nsteps, 128, _RH], f32)
        done = nc.dram_tensor("done", [128, 4], f32, kind="ExternalOutput")
    else:
        outt = nc.dram_tensor("outt", [nsteps, 128, _RH], f32,
                              kind="ExternalOutput")
        done = None

    with tile.TileContext(nc) as tc:
        with (
            tc.tile_pool(name="singles", bufs=1) as singles,
            tc.tile_pool(name="zpool", bufs=3, space="PSUM") as zpool,
            tc.tile_pool(name="ppool", bufs=2, space="PSUM") as ppool,
            tc.tile_pool(name="apool", bufs=4) as apool,
            tc.tile_pool(name="ypool", bufs=6) as ypool,
            tc.tile_pool(name="cpool", bufs=4) as cpool,
        ):
            ybuf = [singles.tile([128, _RH], f32, tag="ybuf0", name="ybuf0"),
                    singles.tile([128, _RH], f32, tag="ybuf1", name="ybuf1")]
            # bf16 mirrors of the state, read only by stage-1 matmuls
            ybufr = [singles.tile([128, _RH], bf16, tag="ybufr0", name="ybufr0"),
                     singles.tile([128, _RH], bf16, tag="ybufr1", name="ybufr1")]
            w1sb = singles.tile([128, _H], bf16, tag="w1sb")
            w2sb = singles.tile([128, n_hslots, 3, 2, _L], bf16, tag="w2sb")
            nc.sync.dma_start(out=ybuf[0][:, :], in_=y0f.ap())
            nc.sync.dma_start(out=ybufr[0][:, :], in_=y0b.ap())
            nc.sync.dma_start(out=w1sb[:, :], in_=w1d.ap())
            nc.sync.dma_start(out=w2sb[:, :, :, :, :], in_=w2d.ap())
            if b1_nonzero:
                b1sb = singles.tile([128, 2], f32, tag="b1sb")
                nc.sync.dma_start(out=b1sb[:, :], in_=b1d.ap())
            if b2_nonzero:
                b2sb = singles.tile([128, n_hslots, 3], f32, tag="b2sb")
                nc.sync.dma_start(out=b2sb[:, :, :], in_=b2d.ap())

            for s in range(nsteps * repeat):
                s = s % nsteps
                slot = 0 if n_hslots == 1 else s
                ycur = ybuf[s % 2]
                ynxt = ybuf[(s + 1) % 2]
                ycurr = ybufr[s % 2]
                ynxtr = ybufr[(s + 1) % 2]
                # Wavefront emission: stage-outer, tiles-inner, so each
                # engine's (in-order) stream holds independent tiles and
                # pipelines fill.
                ysls = [ycur[:, t * _WT:(t + 1) * _WT] for t in range(_NT)]
                prev = [ycurr[:, t * _WT:(t + 1) * _WT] for t in range(_NT)]
                csum = [None] * _NT
                for e in range(4):
                    v = 0 if e < 2 else (1 if e == 2 else 2)
                    for t in range(_NT):
                        ysl = ysls[t]
                        # --- z = W1^T y_e for both row-halves ---
                        zs = []
                        for half in range(2):
                            hp = half * 64
                            z = zpool.tile([128, 2, _WT], f32, tag="z")
                            rhs = prev[t][hp:hp + 64, :]
                            nc.tensor.matmul(z[:, 0],
                                             w1sb[hp:hp + 64, 0:128],
                                             rhs, start=True, stop=True)
                            nc.tensor.matmul(z[:, 1],
                                             w1sb[hp:hp + 64, 128:256],
                                             rhs, start=True, stop=True)
                            zs.append(z)
                        # --- a = tanh(z) ---
                        as_ = []
                        for half in range(2):
                            z = zs[half]
                            a = apool.tile([128, 2, _WT], bf16, tag="a")
                            if b1_nonzero:
                                nc.scalar.activation(a[:, 0], z[:, 0],
                                                     Act.Tanh,
                                                     bias=b1sb[:, 0])
                                nc.scalar.activation(a[:, 1], z[:, 1],
                                                     Act.Tanh,
                                                     bias=b1sb[:, 1])
                            else:
                                nc.scalar.activation(a[:, :, :], z[:, :, :],
                                                     Act.Tanh)
                            as_.append(a)
                        # --- P = (s_e W2)^T a, col-tiled into one PSUM ---
                        p = ppool.tile([128, _WT], f32, tag="p")
                        for half in range(2):
                            a = as_[half]
                            hp = half * 64
                            tp = (0, hp)
                            nc.tensor.matmul(p[hp:hp + 64, :],
                                             w2sb[:, slot, v, 0], a[:, 0],
                                             start=True, stop=False,
                                             tile_position=tp)
                            nc.tensor.matmul(p[hp:hp + 64, :],
                                             w2sb[:, slot, v, 1], a[:, 1],
                                             start=False, stop=True,
                                             tile_position=tp)
                        if e < 3:
                            # y_{e+2} = y + P_e  (bf16, feeds next stage mm)
                            yn = ypool.tile([128, _WT], bf16, tag=f"y{e}")
                            if b2_nonzero:
                                nc.vector.scalar_tensor_tensor(
                                    yn[:, :], p[:, :], b2sb[:, slot, v],
                                    ysl, Alu.add, Alu.add)
                            else:
                                nc.vector.tensor_add(yn[:, :], p[:, :], ysl)
                            prev[t] = yn[:, :]
                            # fp32 running combine:  c = P1 + 2 P2 + P3
                            c = cpool.tile([128, _WT], f32, tag=f"c{e}")
                            if e == 0:
                                if b2_nonzero:
                                    nc.vector.tensor_single_scalar(
                                        c[:, :], p[:, :], b2sb[:, slot, 0],
                                        Alu.add)
                                else:
                                    nc.vector.tensor_copy(c[:, :], p[:, :])
                            elif e == 1:
                                # b2 term folded as 2*(h/2)b2 = h*b2 -> slot 1
                                nc.vector.scalar_tensor_tensor(
                                    c[:, :], p[:, :], 2.0, csum[t],
                                    Alu.mult, Alu.add)
                                if b2_nonzero:
                                    nc.vector.tensor_single_scalar(
                                        c[:, :], c[:, :], b2sb[:, slot, 1],
                                        Alu.add)
                            else:
                                if b2_nonzero:
                                    nc.vector.scalar_tensor_tensor(
                                        c[:, :], p[:, :], b2sb[:, slot, 1],
                                        csum[t], Alu.add, Alu.add)
                                else:
                                    nc.vector.tensor_add(c[:, :], p[:, :],
                                                         csum[t])
                            csum[t] = c[:, :]
                        else:
                            # d = (P1 + 2P2 + P3)/3 + P4
                            d = cpool.tile([128, _WT], f32, tag="d")
                            nc.vector.scalar_tensor_tensor(
                                d[:, :], csum[t], 1.0 / 3.0, p[:, :],
                                Alu.mult, Alu.add)
                            nsl = ynxt[:, t * _WT:(t + 1) * _WT]
                            if b2_nonzero:
                                nc.vector.scalar_tensor_tensor(
                                    nsl, d[:, :], b2sb[:, slot, 2],
                                    ysl, Alu.add, Alu.add)
                            else:
                                nc.vector.tensor_add(nsl, d[:, :], ysl)
                            if s + 1 < nsteps or repeat > 1:
                                nc.vector.tensor_copy(
                                    ynxtr[:, t * _WT:(t + 1) * _WT], nsl)
                nc.sync.dma_start(out=outt.ap()[s], in_=ynxt[:, :])
            if slim:
                nc.sync.dma_start(out=done.ap(), in_=ybuf[0][:, 0:4])

    nc.compile()
    return nc


def _prep_inputs(first_point, time_steps_to_predict, W1, b1, W2, b2):
    """Host-side shard + transpose + weight prescale. Returns (key, in_maps, nsteps)."""
    fp = np.ascontiguousarray(np.asarray(first_point, dtype=np.float32))
    ts = np.asarray(time_steps_to_predict, dtype=np.float32)
    W1 = np.ascontiguousarray(np.asarray(W1, dtype=np.float32))
    W2 = np.ascontiguousarray(np.asarray(W2, dtype=np.float32))
    b1 = np.asarray(b1, dtype=np.float32)
    b2 = np.asarray(b2, dtype=np.float32)

    nsteps = int(ts.shape[0]) - 1
    hs = np.diff(ts.astype(np.float64)).astype(np.float32)      # [nsteps]
    uniform = bool(np.all(hs == hs[0]))
    n_hslots = 1 if uniform else nsteps
    hs_used = hs[:1] if uniform else hs

    b1_nonzero = bool(np.any(b1))
    b2_nonzero = bool(np.any(b2))

    flat = fp.reshape(_ROWS, _L)

    # W1 as bf16 lhsT, duplicated across partition halves: [128, 256]
    w1b = np.ascontiguousarray(np.vstack([W1, W1]).astype(ml_dtypes.bfloat16))
    # W2 as [128 partitions, kblock, 64], scaled per (slot, variant), bf16
    w2kb = W2.reshape(2, 128, _L).transpose(1, 0, 2)            # [128, 2, 64]
    scales = np.stack([hs_used / 2.0, hs_used, hs_used / 6.0], axis=1)  # [S,3]
    w2s = (scales[None, :, :, None, None] *
           w2kb[:, None, None, :, :]).astype(ml_dtypes.bfloat16)
    w2s = np.ascontiguousarray(w2s)                             # [128,S,3,2,64]

    in_maps = []
    for c in range(_NCORES):
        shard = flat[c * _R:(c + 1) * _R]                       # [R, 64]
        y0 = np.empty((128, _RH), np.float32)
        y0[0:64] = shard[0:_RH].T
        y0[64:128] = shard[_RH:].T
        m = {"y0f": y0, "y0b": y0.astype(ml_dtypes.bfloat16),
             "w1d": w1b, "w2d": w2s}
        if b1_nonzero:
            m["b1d"] = np.ascontiguousarray(b1.reshape(2, 128).T)
        if b2_nonzero:
            b2s = np.empty((128, n_hslots, 3), np.float32)
            for half in range(2):
                sl = slice(half * 64, half * 64 + 64)
                b2s[sl, :, 0] = b2[:, None] * (hs_used / 2.0)[None, :]
                b2s[sl, :, 1] = b2[:, None] * hs_used[None, :]
                b2s[sl, :, 2] = b2[:, None] * (hs_used / 6.0)[None, :]
            m["b2d"] = b2s
        in_maps.append(m)

    key = (nsteps, n_hslots, b1_nonzero, b2_nonzero)
    return key, in_maps, nsteps


def get_nc(first_point, time_steps_to_predict, W1, b1, W2, b2):
    """Build (or fetch cached) the compiled Bass program for these inputs."""
    key, in_maps, nsteps = _prep_inputs(
        first_point, time_steps_to_predict, W1, b1, W2, b2)
    if key not in _BUILD_CACHE:
        _BUILD_CACHE[key] = _build(*key)
    return _BUILD_CACHE[key], in_maps, nsteps


def _assemble(first_point, core_outs, nsteps):
    """core_outs: list of [nsteps, 128, RH] per core -> full [1, T, B, N, L]."""
    fp = np.asarray(first_point, dtype=np.float32)
    out = np.empty((_NTRAJ, nsteps + 1, _B, _N, _L), np.float32)
    out[:, 0] = fp
    bs = _B // _NCORES                                          # batches/core
    for c in range(_NCORES):
        dev = core_outs[c]                                      # [S, 128, RH]
        shard = np.concatenate(
            [dev[:, 0:64, :].transpose(0, 2, 1),
             dev[:, 64:128, :].transpose(0, 2, 1)], axis=1)     # [S, R, 64]
        out[0, 1:, c * bs:(c + 1) * bs] = shard.reshape(nsteps, bs, _N, _L)
    return out


def kernel(first_point, time_steps_to_predict, W1, b1, W2, b2):
    from concourse.bass_utils import run_bass_kernel_spmd

    nc, in_maps, nsteps = get_nc(
        first_point, time_steps_to_predict, W1, b1, W2, b2)
    res = run_bass_kernel_spmd(nc, in_maps, core_ids=list(range(_NCORES)))
    core_outs = [res.results[c]["outt"] for c in range(_NCORES)]
    return _assemble(first_point, core_outs, nsteps)


# revision 16
# speedup vs baseline: 7995.0504x; 6354.8116x over previous
"""Trainium2 Bass kernel for nn_DiffeqSolver (RK4 ODE solve, 2-layer tanh MLP drift).

Strategy (data-parallel across 8 NeuronCores):
  - Shard the 32768 latent rows (NTRAJ*B*N) across 8 cores -> 4096 rows/core.
  - On-chip everything is feature-major: y^T [64, rows].  Rows are split into
    two halves packed on SBUF partitions 0-63 (rows 0..2047) and 64-127
    (rows 2048..4095), so elementwise ops run on all 128 lanes and the two
    halves' matmuls run concurrently in separate PE row/column groups.
  - Matmuls are bf16 (weights + stage inputs); PSUM accumulation and the
    persistent state stay fp32.  bf16 rounding only enters through the
    h-scaled drift k_i, so state error stays ~1e-4/step.
  - Per RK4 stage i: z = W1^T y_i^T (row-tiled pairs, PSUM [128,2,512] per
    half), a = tanh(z) (one wide ACT op per half), P_i = (s_i W2)^T a
    (col-tiled pairs accumulating into one PSUM [128,512] tile) with RK4
    factors s_i = (h/2, h/2, h, h/6) folded into host-prescaled W2 copies.
  - y_{i+1} = y + P_i (one DVE op, bf16 out, feeds next stage's matmul).
    Final combine in full fp32 from the PSUMs:
      y_next = y + (P1 + 2 P2 + P3)/3 + P4.
  - Output is written transposed ([steps, 128, 2048] per core); the host
    unpacks halves and re-transposes while gathering.
"""

import sys

if "/opt/trn_rl_repo" not in sys.path:
    sys.path.insert(0, "/opt/trn_rl_repo")

import numpy as np
import ml_dtypes

_NCORES = 8
_T = 32
_NTRAJ, _B, _N, _L = 1, 32, 1024, 64
_H = 256
_ROWS = _NTRAJ * _B * _N          # 32768 total latent rows
_R = _ROWS // _NCORES             # 4096 rows per core
_RH = _R // 2                     # 2048 rows per partition-half
_WT = 512                         # column-tile width (matmul moving-dim)
_NT = _RH // _WT                  # 4 column tiles per step
_SWP = 3                          # software-pipeline depth (tiles)
_ZB, _PB, _AB, _CB = 3, 2, 8, 6   # pool depths (z/p PSUM banks: 2*_ZB + _PB <= 8)

_BUILD_CACHE = {}


def _build(nsteps: int, n_hslots: int, b1_nonzero: bool, b2_nonzero: bool,
           repeat: int = 1, slim: bool = False, ablate: frozenset = frozenset()):
    import concourse.mybir as mybir
    import concourse.tile as tile
    from concourse import bacc

    f32 = mybir.dt.float32
    bf16 = mybir.dt.bfloat16
    Alu = mybir.AluOpType
    Act = mybir.ActivationFunctionType

    nc = bacc.Bacc("TRN2", target_bir_lowering=False, debug=False,
                   num_devices=_NCORES)

    y0f = nc.dram_tensor("y0f", [128, _RH], f32, kind="ExternalInput")
    y0b = nc.dram_tensor("y0b", [128, _RH], bf16, kind="ExternalInput")
    w1d = nc.dram_tensor("w1d", [128, _H], bf16, kind="ExternalInput")
    # Host-prescaled W2 variants: [128, slot, variant(h/2, h, h/6), kblock, 64]
    w2d = nc.dram_tensor("w2d", [128, n_hslots, 3, 2, _L], bf16,
                         kind="ExternalInput")
    b1d = (nc.dram_tensor("b1d", [128, 2], f32, kind="ExternalInput")
           if b1_nonzero else None)
    # b2 scaled by (h/2, h, h/6) per variant, partition-halves duplicated
    b2d = (nc.dram_tensor("b2d", [128, n_hslots, 3], f32, kind="ExternalInput")
           if b2_nonzero else None)
    if slim:
        outt = nc.dram_tensor("outt", [nsteps, 128, _RH], f32)
        done = nc.dram_tensor("done", [128, 4], f32, kind="ExternalOutput")
    else:
        outt = nc.dram_tensor("outt", [nsteps, 128, _RH], f32,
                              kind="ExternalOutput")
        done = None

    with tile.TileContext(nc) as tc:
        with (
            tc.tile_pool(name="singles", bufs=1) as singles,
            tc.tile_pool(name="zpool", bufs=_ZB, space="PSUM") as zpool,
            tc.tile_pool(name="ppool", bufs=_PB, space="PSUM") as ppool,
            tc.tile_pool(name="apool", bufs=_AB) as apool,
            tc.tile_pool(name="ypool", bufs=6) as ypool,
            tc.tile_pool(name="cpool", bufs=_CB) as cpool,
        ):
            ybuf = [singles.tile([128, _RH], f32, tag="ybuf0", name="ybuf0"),
                    singles.tile([128, _RH], f32, tag="ybuf1", name="ybuf1")]
            # bf16 mirrors of the state, read only by stage-1 matmuls
            ybufr = [singles.tile([128, _RH], bf16, tag="ybufr0", name="ybufr0"),
                     singles.tile([128, _RH], bf16, tag="ybufr1", name="ybufr1")]
            w1sb = singles.tile([128, _H], bf16, tag="w1sb")
            adummy = singles.tile([128, 2, _WT], bf16, tag="adummy")
            nc.vector.memset(adummy[:, :, :], 0.25)
            w2sb = singles.tile([128, n_hslots, 3, 2, _L], bf16, tag="w2sb")
            nc.sync.dma_start(out=ybuf[0][:, :], in_=y0f.ap())
            nc.sync.dma_start(out=ybufr[0][:, :], in_=y0b.ap())
            nc.sync.dma_start(out=w1sb[:, :], in_=w1d.ap())
            nc.sync.dma_start(out=w2sb[:, :, :, :, :], in_=w2d.ap())
            if b1_nonzero:
                b1sb = singles.tile([128, 2], f32, tag="b1sb")
                nc.sync.dma_start(out=b1sb[:, :], in_=b1d.ap())
            if b2_nonzero:
                b2sb = singles.tile([128, n_hslots, 3], f32, tag="b2sb")
                nc.sync.dma_start(out=b2sb[:, :, :], in_=b2d.ap())

            for s in range(nsteps * repeat):
                s = s % nsteps
                slot = 0 if n_hslots == 1 else s
                if ablate:
                    ycur = ynxt = ybuf[0]
                    ycurr = ynxtr = ybufr[0]
                else:
                    ycur = ybuf[s % 2]
                    ynxt = ybuf[(s + 1) % 2]
                    ycurr = ybufr[s % 2]
                    ynxtr = ybufr[(s + 1) % 2]
                # Wavefront emission: stage-outer, tiles-inner, so each
                # engine's (in-order) stream holds independent tiles and
                # pipelines fill.
                ysls = [ycur[:, t * _WT:(t + 1) * _WT] for t in range(_NT)]
                prev = [ycurr[:, t * _WT:(t + 1) * _WT] for t in range(_NT)]
                csum = [None] * _NT
                for e in range(4):
                    v = 0 if e < 2 else (1 if e == 2 else 2)
                    amem = [None] * _NT

                    def stage_a(t, e=e, amem=amem, prev=prev):
                        # z = W1^T y_e (row-tiled halves) ; a = tanh(z)
                        as_ = []
                        for half in range(2):
                            hp = half * 64
                            if 'mm1' not in ablate:
                                z = zpool.tile([128, 2, _WT], f32, tag="z",
                                               name="z")
                                rhs = prev[t][hp:hp + 64, :]
                                nc.tensor.matmul(z[:, 0],
                                                 w1sb[hp:hp + 64, 0:128],
                                                 rhs, start=True, stop=True)
                                nc.tensor.matmul(z[:, 1],
                                                 w1sb[hp:hp + 64, 128:256],
                                                 rhs, start=True, stop=True)
                            if 'act' in ablate or 'mm1' in ablate:
                                as_.append(adummy)
                                continue
                            a = apool.tile([128, 2, _WT], bf16, tag="a",
                                           name="a")
                            if b1_nonzero:
                                nc.scalar.activation(a[:, 0], z[:, 0],
                                                     Act.Tanh,
                                                     bias=b1sb[:, 0])
                                nc.scalar.activation(a[:, 1], z[:, 1],
                                                     Act.Tanh,
                                                     bias=b1sb[:, 1])
                            else:
                                nc.scalar.activation(a[:, :, :], z[:, :, :],
                                                     Act.Tanh)
                            as_.append(a)
                        amem[t] = as_

                    def stage_b(t, e=e, v=v, s=s, amem=amem, prev=prev,
                                csum=csum, ynxt=ynxt, ynxtr=ynxtr):
                        if 'mm2' in ablate:
                            return
                        ysl = ysls[t]
                        as_ = amem[t]
                        p = ppool.tile([128, _WT], f32, tag="p", name="p")
                        for half in range(2):
                            a = as_[half]
                            hp = half * 64
                            tp = (0, hp)
                            nc.tensor.matmul(p[hp:hp + 64, :],
                                             w2sb[:, slot, v, 0], a[:, 0],
                                             start=True, stop=False,
                                             tile_position=tp)
                            nc.tensor.matmul(p[hp:hp + 64, :],
                                             w2sb[:, slot, v, 1], a[:, 1],
                                             start=False, stop=True,
                                             tile_position=tp)
                        if 'dve' in ablate:
                            return
                        if e < 3:
                            # y_{e+2} = y + P_e  (bf16, feeds next stage mm)
                            yn = ypool.tile([128, _WT], bf16, tag=f"y{e}",
                                            name="yn")
                            if b2_nonzero:
                                nc.vector.scalar_tensor_tensor(
                                    yn[:, :], p[:, :], b2sb[:, slot, v],
                                    ysl, Alu.add, Alu.add)
                            else:
                                nc.vector.tensor_add(yn[:, :], p[:, :], ysl)
                            prev[t] = yn[:, :]
                            # fp32 running combine:  c = P1 + 2 P2 + P3
                            c = cpool.tile([128, _WT], f32, tag=f"c{e}",
                                           name="c")
                            if e == 0:
                                if b2_nonzero:
                                    nc.vector.tensor_single_scalar(
                                        c[:, :], p[:, :], b2sb[:, slot, 0],
                                        Alu.add)
                                else:
                                    nc.vector.tensor_copy(c[:, :], p[:, :])
                            elif e == 1:
                                nc.vector.scalar_tensor_tensor(
                                    c[:, :], p[:, :], 2.0, csum[t],
                                    Alu.mult, Alu.add)
                                if b2_nonzero:
                                    nc.vector.tensor_single_scalar(
                                        c[:, :], c[:, :], b2sb[:, slot, 1],
                                        Alu.add)
                            else:
                                if b2_nonzero:
                                    nc.vector.scalar_tensor_tensor(
                                        c[:, :], p[:, :], b2sb[:, slot, 1],
                                        csum[t], Alu.add, Alu.add)
                                else:
                                    nc.vector.tensor_add(c[:, :], p[:, :],
                                                         csum[t])
                            csum[t] = c[:, :]
                        else:
                            # d = (P1 + 2P2 + P3)/3 + P4
                            d = cpool.tile([128, _WT], f32, tag="d", name="d")
                            nc.vector.scalar_tensor_tensor(
                                d[:, :], csum[t], 1.0 / 3.0, p[:, :],
                                Alu.mult, Alu.add)
                            nsl = ynxt[:, t * _WT:(t + 1) * _WT]
                            if b2_nonzero:
                                nc.vector.scalar_tensor_tensor(
                                    nsl, d[:, :], b2sb[:, slot, 2],
                                    ysl, Alu.add, Alu.add)
                            else:
                                nc.vector.tensor_add(nsl, d[:, :], ysl)
                            if s + 1 < nsteps or repeat > 1:
                                nc.vector.tensor_copy(
                                    ynxtr[:, t * _WT:(t + 1) * _WT], nsl)

                    # software-pipelined emission: stage_b lags by _SWP tiles
                    for t in range(_NT + _SWP):
                        if t < _NT:
                            stage_a(t)
                        if t >= _SWP:
                            stage_b(t - _SWP)
                nc.sync.dma_start(out=outt.ap()[s], in_=ycur[:, :]
                                  if ablate else ynxt[:, :])
            if slim:
                nc.sync.dma_start(out=done.ap(), in_=ybuf[0][:, 0:4])

    nc.compile()
    return nc


def _prep_inputs(first_point, time_steps_to_predict, W1, b1, W2, b2):
    """Host-side shard + transpose + weight prescale. Returns (key, in_maps, nsteps)."""
    fp = np.ascontiguousarray(np.asarray(first_point, dtype=np.float32))
    ts = np.asarray(time_steps_to_predict, dtype=np.float32)
    W1 = np.ascontiguousarray(np.asarray(W1, dtype=np.float32))
    W2 = np.ascontiguousarray(np.asarray(W2, dtype=np.float32))
    b1 = np.asarray(b1, dtype=np.float32)
    b2 = np.asarray(b2, dtype=np.float32)

    nsteps = int(ts.shape[0]) - 1
    hs = np.diff(ts.astype(np.float64)).astype(np.float32)      # [nsteps]
    uniform = bool(np.all(hs == hs[0]))
    n_hslots = 1 if uniform else nsteps
    hs_used = hs[:1] if uniform else hs

    b1_nonzero = bool(np.any(b1))
    b2_nonzero = bool(np.any(b2))

    flat = fp.reshape(_ROWS, _L)

    # W1 as bf16 lhsT, duplicated across partition halves: [128, 256]
    w1b = np.ascontiguousarray(np.vstack([W1, W1]).astype(ml_dtypes.bfloat16))
    # W2 as [128 partitions, kblock, 64], scaled per (slot, variant), bf16
    w2kb = W2.reshape(2, 128, _L).transpose(1, 0, 2)            # [128, 2, 64]
    scales = np.stack([hs_used / 2.0, hs_used, hs_used / 6.0], axis=1)  # [S,3]
    w2s = (scales[None, :, :, None, None] *
           w2kb[:, None, None, :, :]).astype(ml_dtypes.bfloat16)
    w2s = np.ascontiguousarray(w2s)                             # [128,S,3,2,64]

    in_maps = []
    for c in range(_NCORES):
        shard = flat[c * _R:(c + 1) * _R]                       # [R, 64]
        y0 = np.empty((128, _RH), np.float32)
        y0[0:64] = shard[0:_RH].T
        y0[64:128] = shard[_RH:].T
        m = {"y0f": y0, "y0b": y0.astype(ml_dtypes.bfloat16),
             "w1d": w1b, "w2d": w2s}
        if b1_nonzero:
            m["b1d"] = np.ascontiguousarray(b1.reshape(2, 128).T)
        if b2_nonzero:
            b2s = np.empty((128, n_hslots, 3), np.float32)
            for half in range(2):
                sl = slice(half * 64, half * 64 + 64)
                b2s[sl, :, 0] = b2[:, None] * (hs_used / 2.0)[None, :]
                b2s[sl, :, 1] = b2[:, None] * hs_used[None, :]
                b2s[sl, :, 2] = b2[:, None] * (hs_used / 6.0)[None, :]
            m["b2d"] = b2s
        in_maps.append(m)

    key = (nsteps, n_hslots, b1_nonzero, b2_nonzero)
    return key, in_maps, nsteps


def get_nc(first_point, time_steps_to_predict, W1, b1, W2, b2):
    """Build (or fetch cached) the compiled Bass program for these inputs."""
    key, in_maps, nsteps = _prep_inputs(
        first_point, time_steps_to_predict, W1, b1, W2, b2)
    if key not in _BUILD_CACHE:
        _BUILD_CACHE[key] = _build(*key)
    return _BUILD_CACHE[key], in_maps, nsteps


def _assemble(first_point, core_outs, nsteps):
    """core_outs: list of [nsteps, 128, RH] per core -> full [1, T, B, N, L]."""
    fp = np.asarray(first_point, dtype=np.float32)
    out = np.empty((_NTRAJ, nsteps + 1, _B, _N, _L), np.float32)
    out[:, 0] = fp
    bs = _B // _NCORES                                          # batches/core
    for c in range(_NCORES):
        dev = core_outs[c]                                      # [S, 128, RH]
        shard = np.concatenate(
            [dev[:, 0:64, :].transpose(0, 2, 1),
             dev[:, 64:128, :].transpose(0, 2, 1)], axis=1)     # [S, R, 64]
        out[0, 1:, c * bs:(c + 1) * bs] = shard.reshape(nsteps, bs, _N, _L)
    return out


def kernel(first_point, time_steps_to_predict, W1, b1, W2, b2):
    from concourse.bass_utils import run_bass_kernel_spmd

    nc, in_maps, nsteps = get_nc(
        first_point, time_steps_to_predict, W1, b1, W2, b2)
    res = run_bass_kernel_spmd(nc, in_maps, core_ids=list(range(_NCORES)))
    core_outs = [res.results[c]["outt"] for c in range(_NCORES)]
    return _assemble(first_point, core_outs, nsteps)


# revision 17
# speedup vs baseline: 31699.3677x; 3.9649x over previous
"""Trainium2 Bass kernel for nn_DiffeqSolver (RK4 ODE solve, 2-layer tanh MLP drift).

Strategy (data-parallel across 8 NeuronCores):
  - Shard the 32768 latent rows (NTRAJ*B*N) across 8 cores -> 4096 rows/core.
  - On-chip everything is feature-major: y^T [64, rows].  Rows are split into
    two halves packed on SBUF partitions 0-63 (rows 0..2047) and 64-127
    (rows 2048..4095), so elementwise ops run on all 128 lanes and the two
    halves' matmuls run concurrently in separate PE row/column groups.
  - Matmuls are bf16 (weights + stage inputs); PSUM accumulation and the
    persistent state stay fp32.  bf16 rounding only enters through the
    h-scaled drift k_i, so state error stays ~1e-4/step.
  - Per RK4 stage i: z = W1^T y_i^T (row-tiled pairs, PSUM [128,2,512] per
    half), a = tanh(z) (one wide ACT op per half), P_i = (s_i W2)^T a
    (col-tiled pairs accumulating into one PSUM [128,512] tile) with RK4
    factors s_i = (h/2, h/2, h, h/6) folded into host-prescaled W2 copies.
  - y_{i+1} = y + P_i (one DVE op, bf16 out, feeds next stage's matmul).
    Final combine in full fp32 from the PSUMs:
      y_next = y + (P1 + 2 P2 + P3)/3 + P4.
  - Output is written transposed ([steps, 128, 2048] per core); the host
    unpacks halves and re-transposes while gathering.
"""

import sys

if "/opt/trn_rl_repo" not in sys.path:
    sys.path.insert(0, "/opt/trn_rl_repo")

import numpy as np
import ml_dtypes

_NCORES = 8
_T = 32
_NTRAJ, _B, _N, _L = 1, 32, 1024, 64
_H = 256
_ROWS = _NTRAJ * _B * _N          # 32768 total latent rows
_R = _ROWS // _NCORES             # 4096 rows per core
_RH = _R // 2                     # 2048 rows per partition-half
_WT = 512                         # column-tile width (matmul moving-dim)
_NT = _RH // _WT                  # 4 column tiles per step
_SWP = 3                          # software-pipeline depth (tiles)
_ZB, _PB, _AB, _CB = 3, 2, 8, 6   # pool depths (z/p PSUM banks: 2*_ZB + _PB <= 8)

_BUILD_CACHE = {}


def _build(nsteps: int, n_hslots: int, b1_nonzero: bool, b2_nonzero: bool,
           repeat: int = 1, slim: bool = False, ablate: frozenset = frozenset()):
    import concourse.mybir as mybir
    import concourse.tile as tile
    from concourse import bacc

    f32 = mybir.dt.float32
    bf16 = mybir.dt.bfloat16
    Alu = mybir.AluOpType
    Act = mybir.ActivationFunctionType

    nc = bacc.Bacc("TRN2", target_bir_lowering=False, debug=False,
                   num_devices=_NCORES)

    y0f = nc.dram_tensor("y0f", [128, _RH], f32, kind="ExternalInput")
    y0b = nc.dram_tensor("y0b", [128, _RH], bf16, kind="ExternalInput")
    w1d = nc.dram_tensor("w1d", [128, _H], bf16, kind="ExternalInput")
    # Host-prescaled W2 variants: [128, slot, variant(h/2, h, h/6), kblock, 64]
    w2d = nc.dram_tensor("w2d", [128, n_hslots, 3, 2, _L], bf16,
                         kind="ExternalInput")
    b1d = (nc.dram_tensor("b1d", [128, 2], f32, kind="ExternalInput")
           if b1_nonzero else None)
    # b2 scaled by (h/2, h, h/6) per variant, partition-halves duplicated
    b2d = (nc.dram_tensor("b2d", [128, n_hslots, 3], f32, kind="ExternalInput")
           if b2_nonzero else None)
    if slim:
        outt = nc.dram_tensor("outt", [nsteps, 128, _RH], f32)
        done = nc.dram_tensor("done", [128, 4], f32, kind="ExternalOutput")
    else:
        outt = nc.dram_tensor("outt", [nsteps, 128, _RH], f32,
                              kind="ExternalOutput")
        done = None

    with tile.TileContext(nc) as tc:
        with (
            tc.tile_pool(name="singles", bufs=1) as singles,
            tc.tile_pool(name="zpool", bufs=_ZB, space="PSUM") as zpool,
            tc.tile_pool(name="ppool", bufs=_PB, space="PSUM") as ppool,
            tc.tile_pool(name="apool", bufs=_AB) as apool,
            tc.tile_pool(name="ypool", bufs=6) as ypool,
            tc.tile_pool(name="cpool", bufs=_CB) as cpool,
        ):
            ybuf = [singles.tile([128, _RH], f32, tag="ybuf0", name="ybuf0"),
                    singles.tile([128, _RH], f32, tag="ybuf1", name="ybuf1")]
            # bf16 mirrors of the state, read only by stage-1 matmuls
            ybufr = [singles.tile([128, _RH], bf16, tag="ybufr0", name="ybufr0"),
                     singles.tile([128, _RH], bf16, tag="ybufr1", name="ybufr1")]
            w1sb = singles.tile([128, _H], bf16, tag="w1sb")
            adummy = singles.tile([128, 2, _WT], bf16, tag="adummy")
            nc.vector.memset(adummy[:, :, :], 0.25)
            w2sb = singles.tile([128, n_hslots, 3, 2, _L], bf16, tag="w2sb")
            nc.sync.dma_start(out=ybuf[0][:, :], in_=y0f.ap())
            nc.sync.dma_start(out=ybufr[0][:, :], in_=y0b.ap())
            nc.sync.dma_start(out=w1sb[:, :], in_=w1d.ap())
            nc.sync.dma_start(out=w2sb[:, :, :, :, :], in_=w2d.ap())
            if b1_nonzero:
                b1sb = singles.tile([128, 2], f32, tag="b1sb")
                nc.sync.dma_start(out=b1sb[:, :], in_=b1d.ap())
            if b2_nonzero:
                b2sb = singles.tile([128, n_hslots, 3], f32, tag="b2sb")
                nc.sync.dma_start(out=b2sb[:, :, :], in_=b2d.ap())

            for s in range(nsteps * repeat):
                s = s % nsteps
                slot = 0 if n_hslots == 1 else s
                if ablate:
                    ycur = ynxt = ybuf[0]
                    ycurr = ynxtr = ybufr[0]
                else:
                    ycur = ybuf[s % 2]
                    ynxt = ybuf[(s + 1) % 2]
                    ycurr = ybufr[s % 2]
                    ynxtr = ybufr[(s + 1) % 2]
                # Wavefront emission: stage-outer, tiles-inner, so each
                # engine's (in-order) stream holds independent tiles and
                # pipelines fill.
                ysls = [ycur[:, t * _WT:(t + 1) * _WT] for t in range(_NT)]
                prev = [ycurr[:, t * _WT:(t + 1) * _WT] for t in range(_NT)]
                csum = [None] * _NT
                for e in range(4):
                    v = 0 if e < 2 else (1 if e == 2 else 2)
                    amem = [None] * _NT

                    def stage_a(t, e=e, amem=amem, prev=prev):
                        # z = W1^T y_e (row-tiled halves) ; a = tanh(z)
                        as_ = []
                        for half in range(2):
                            hp = half * 64
                            if 'mm1' not in ablate:
                                z = zpool.tile([128, 2, _WT], f32, tag="z",
                                               name="z")
                                rhs = prev[t][hp:hp + 64, :]
                                nc.tensor.matmul(z[:, 0],
                                                 w1sb[hp:hp + 64, 0:128],
                                                 rhs, start=True, stop=True)
                                nc.tensor.matmul(z[:, 1],
                                                 w1sb[hp:hp + 64, 128:256],
                                                 rhs, start=True, stop=True)
                            if 'act' in ablate or 'mm1' in ablate:
                                as_.append(adummy)
                                continue
                            a = apool.tile([128, 2, _WT], bf16, tag="a",
                                           name="a")
                            if b1_nonzero:
                                nc.scalar.activation(a[:, 0], z[:, 0],
                                                     Act.Tanh,
                                                     bias=b1sb[:, 0:1])
                                nc.scalar.activation(a[:, 1], z[:, 1],
                                                     Act.Tanh,
                                                     bias=b1sb[:, 1:2])
                            else:
                                nc.scalar.activation(a[:, :, :], z[:, :, :],
                                                     Act.Tanh)
                            as_.append(a)
                        amem[t] = as_

                    def stage_b(t, e=e, v=v, s=s, amem=amem, prev=prev,
                                csum=csum, ynxt=ynxt, ynxtr=ynxtr):
                        if 'mm2' in ablate:
                            return
                        ysl = ysls[t]
                        as_ = amem[t]
                        p = ppool.tile([128, _WT], f32, tag="p", name="p")
                        for half in range(2):
                            a = as_[half]
                            hp = half * 64
                            tp = (0, hp)
                            nc.tensor.matmul(p[hp:hp + 64, :],
                                             w2sb[:, slot, v, 0], a[:, 0],
                                             start=True, stop=False,
                                             tile_position=tp)
                            nc.tensor.matmul(p[hp:hp + 64, :],
                                             w2sb[:, slot, v, 1], a[:, 1],
                                             start=False, stop=True,
                                             tile_position=tp)
                        if 'dve' in ablate:
                            return
                        if e < 3:
                            # y_{e+2} = y + P_e  (bf16, feeds next stage mm)
                            yn = ypool.tile([128, _WT], bf16, tag=f"y{e}",
                                            name="yn")
                            if b2_nonzero:
                                nc.vector.scalar_tensor_tensor(
                                    yn[:, :], p[:, :], b2sb[:, slot, v:v + 1],
                                    ysl, Alu.add, Alu.add)
                            else:
                                nc.vector.tensor_add(yn[:, :], p[:, :], ysl)
                            prev[t] = yn[:, :]
                            # fp32 running combine:  c = P1 + 2 P2 + P3
                            c = cpool.tile([128, _WT], f32, tag=f"c{e}",
                                           name="c")
                            if e == 0:
                                if b2_nonzero:
                                    nc.vector.tensor_single_scalar(
                                        c[:, :], p[:, :], b2sb[:, slot, 0:1],
                                        Alu.add)
                                else:
                                    nc.vector.tensor_copy(c[:, :], p[:, :])
                            elif e == 1:
                                nc.vector.scalar_tensor_tensor(
                                    c[:, :], p[:, :], 2.0, csum[t],
                                    Alu.mult, Alu.add)
                                if b2_nonzero:
                                    nc.vector.tensor_single_scalar(
                                        c[:, :], c[:, :], b2sb[:, slot, 1:2],
                                        Alu.add)
                            else:
                                if b2_nonzero:
                                    nc.vector.scalar_tensor_tensor(
                                        c[:, :], p[:, :], b2sb[:, slot, 1:2],
                                        csum[t], Alu.add, Alu.add)
                                else:
                                    nc.vector.tensor_add(c[:, :], p[:, :],
                                                         csum[t])
                            csum[t] = c[:, :]
                        else:
                            # d = (P1 + 2P2 + P3)/3 + P4
                            d = cpool.tile([128, _WT], f32, tag="d", name="d")
                            nc.vector.scalar_tensor_tensor(
                                d[:, :], csum[t], 1.0 / 3.0, p[:, :],
                                Alu.mult, Alu.add)
                            nsl = ynxt[:, t * _WT:(t + 1) * _WT]
                            if b2_nonzero:
                                nc.vector.scalar_tensor_tensor(
                                    nsl, d[:, :], b2sb[:, slot, 2:3],
                                    ysl, Alu.add, Alu.add)
                            else:
                                nc.vector.tensor_add(nsl, d[:, :], ysl)
                            if s + 1 < nsteps or repeat > 1:
                                nc.vector.tensor_copy(
                                    ynxtr[:, t * _WT:(t + 1) * _WT], nsl)

                    # software-pipelined emission: stage_b lags by _SWP tiles
                    for t in range(_NT + _SWP):
                        if t < _NT:
                            stage_a(t)
                        if t >= _SWP:
                            stage_b(t - _SWP)
                nc.sync.dma_start(out=outt.ap()[s], in_=ycur[:, :]
                                  if ablate else ynxt[:, :])
            if slim:
                nc.sync.dma_start(out=done.ap(), in_=ybuf[0][:, 0:4])

    nc.compile()
    return nc


def _prep_inputs(first_point, time_steps_to_predict, W1, b1, W2, b2):
    """Host-side shard + transpose + weight prescale. Returns (key, in_maps, nsteps)."""
    fp = np.ascontiguousarray(np.asarray(first_point, dtype=np.float32))
    ts = np.asarray(time_steps_to_predict, dtype=np.float32)
    W1 = np.ascontiguousarray(np.asarray(W1, dtype=np.float32))
    W2 = np.ascontiguousarray(np.asarray(W2, dtype=np.float32))
    b1 = np.asarray(b1, dtype=np.float32)
    b2 = np.asarray(b2, dtype=np.float32)

    nsteps = int(ts.shape[0]) - 1
    hs = np.diff(ts.astype(np.float64)).astype(np.float32)      # [nsteps]
    uniform = bool(np.all(hs == hs[0]))
    n_hslots = 1 if uniform else nsteps
    hs_used = hs[:1] if uniform else hs

    b1_nonzero = bool(np.any(b1))
    b2_nonzero = bool(np.any(b2))

    flat = fp.reshape(_ROWS, _L)

    # W1 as bf16 lhsT, duplicated across partition halves: [128, 256]
    w1b = np.ascontiguousarray(np.vstack([W1, W1]).astype(ml_dtypes.bfloat16))
    # W2 as [128 partitions, kblock, 64], scaled per (slot, variant), bf16
    w2kb = W2.reshape(2, 128, _L).transpose(1, 0, 2)            # [128, 2, 64]
    scales = np.stack([hs_used / 2.0, hs_used, hs_used / 6.0], axis=1)  # [S,3]
    w2s = (scales[None, :, :, None, None] *
           w2kb[:, None, None, :, :]).astype(ml_dtypes.bfloat16)
    w2s = np.ascontiguousarray(w2s)                             # [128,S,3,2,64]

    in_maps = []
    for c in range(_NCORES):
        shard = flat[c * _R:(c + 1) * _R]                       # [R, 64]
        y0 = np.empty((128, _RH), np.float32)
        y0[0:64] = shard[0:_RH].T
        y0[64:128] = shard[_RH:].T
        m = {"y0f": y0, "y0b": y0.astype(ml_dtypes.bfloat16),
             "w1d": w1b, "w2d": w2s}
        if b1_nonzero:
            m["b1d"] = np.ascontiguousarray(b1.reshape(2, 128).T)
        if b2_nonzero:
            b2s = np.empty((128, n_hslots, 3), np.float32)
            for half in range(2):
                sl = slice(half * 64, half * 64 + 64)
                b2s[sl, :, 0] = b2[:, None] * (hs_used / 2.0)[None, :]
                b2s[sl, :, 1] = b2[:, None] * hs_used[None, :]
                b2s[sl, :, 2] = b2[:, None] * (hs_used / 6.0)[None, :]
            m["b2d"] = b2s
        in_maps.append(m)

    key = (nsteps, n_hslots, b1_nonzero, b2_nonzero)
    return key, in_maps, nsteps


def get_nc(first_point, time_steps_to_predict, W1, b1, W2, b2):
    """Build (or fetch cached) the compiled Bass program for these inputs."""
    key, in_maps, nsteps = _prep_inputs(
        first_point, time_steps_to_predict, W1, b1, W2, b2)
    if key not in _BUILD_CACHE:
        _BUILD_CACHE[key] = _build(*key)
    return _BUILD_CACHE[key], in_maps, nsteps


def _assemble(first_point, core_outs, nsteps):
    """core_outs: list of [nsteps, 128, RH] per core -> full [1, T, B, N, L]."""
    fp = np.asarray(first_point, dtype=np.float32)
    out = np.empty((_NTRAJ, nsteps + 1, _B, _N, _L), np.float32)
    out[:, 0] = fp
    bs = _B // _NCORES                                          # batches/core
    for c in range(_NCORES):
        dev = core_outs[c]                                      # [S, 128, RH]
        shard = np.concatenate(
            [dev[:, 0:64, :].transpose(0, 2, 1),
             dev[:, 64:128, :].transpose(0, 2, 1)], axis=1)     # [S, R, 64]
        out[0, 1:, c * bs:(c + 1) * bs] = shard.reshape(nsteps, bs, _N, _L)
    return out


def kernel(first_point, time_steps_to_predict, W1, b1, W2, b2):
    from concourse.bass_utils import run_bass_kernel_spmd

    nc, in_maps, nsteps = get_nc(
        first_point, time_steps_to_predict, W1, b1, W2, b2)
    res = run_bass_kernel_spmd(nc, in_maps, core_ids=list(range(_NCORES)))
    core_outs = [res.results[c]["outt"] for c in range(_NCORES)]
    return _assemble(first_point, core_outs, nsteps)
